# revision 27
# baseline (speedup 1.0000x reference)
"""ArcFace multi-head-sharded loss on 8 TRN2 NeuronCores.

Strategy: shard the (64, 2048, 256) weight table over the group axis —
each core owns 8 groups. Samples are routed host-side to the core owning
their group (the host routing replaces the all-to-all). The host also
pre-normalizes weight rows (cos is scale-invariant in w, so w/||w|| is a
pure re-layout), scales by 16 and quantizes to fp8e4 — this halves HBM
traffic vs bf16 and enables DoubleRow matmuls (contraction of 256 = E in
a single pass, 2 fp8 elements per PE beat).

Each core:
  - streams its 8 pre-normalized weight groups (fp8, 512KB/band),
  - computes cos(b, c) = <x_b, w_c> via DoubleRow matmuls into PSUM
    (samples on PSUM partitions, classes on free dim),
  - extracts the target logit with a tiny per-band matmul against
    host-gathered target columns + diagonal mask,
  - applies the ArcFace margin (sqrt via exp(0.5 ln)) and the CE loss
    per sample on-device: exp with fused per-sample scale (folding
    1/||x||) and accumulation over classes, LSE correction for the
    margin target, weighted reduce to a single scalar via matmul,
  - returns one partial-loss scalar.

Host: sums the 8 scalars. Samples are packed into bands of NG=32
partition rows, one weight group per band, BPT=4 bands per 128-row tile.
"""

import sys
import numpy as np
import ml_dtypes

FP8 = ml_dtypes.float8_e4m3
BF16 = ml_dtypes.bfloat16

_TRN_REPO = "/opt/trn_rl_repo"
if _TRN_REPO not in sys.path:
    sys.path.insert(0, _TRN_REPO)

# problem config (hardcoded per spec)
B, E, G, C = 512, 256, 64, 2048
NCORES = 8
GPC = G // NCORES        # weight groups per core
NG = 32                  # sample slots per band
BPT = 128 // NG          # bands per 128-partition sample tile
NCC = C // 512           # 512-col psum chunks per group
SCALE = 64.0
MARGIN = 0.5
COS_M = float(np.cos(MARGIN))
SIN_M = float(np.sin(MARGIN))
THETA = float(np.cos(np.pi - MARGIN))
SINMM = float(np.sin(np.pi - MARGIN) * MARGIN)

_graph_cache = {}


def _build(nb):
    """Build the per-core Bass graph for nb weight bands (nb % BPT == 0)."""
    from contextlib import ExitStack
    import concourse.bacc as bacc
    import concourse.tile as tile
    from concourse import mybir

    f32 = mybir.dt.float32
    bf16 = mybir.dt.bfloat16
    fp8 = mybir.dt.float8e4
    i32 = mybir.dt.int32
    A = mybir.AluOpType
    AF = mybir.ActivationFunctionType
    DR = mybir.MatmulPerfMode.DoubleRow

    T = nb // BPT
    nc = bacc.Bacc(None)

    wt_ext = nc.declare_dram_parameter("wt", [nb, 128, 2, C], fp8, isOutput=False)
    # xtw packs xt (cols 0..128T) and wtar (cols 128T..256T) in one transfer
    xtw_ext = nc.declare_dram_parameter("xtw", [128, 2, 256 * T], fp8, isOutput=False)
    idn_ext = nc.declare_dram_parameter("idn", [128, 128], bf16, isOutput=False)
    # scal columns: [sc4_0..sc4_{T-1} | rx16_* | redw_*]
    scal_ext = nc.declare_dram_parameter("scal", [128, 3 * T], f32, isOutput=False)
    out_ext = nc.declare_dram_parameter("out", [1, 1], f32, isOutput=True)

    with tile.TileContext(nc) as tc, ExitStack() as ctx:
        wpool = ctx.enter_context(tc.tile_pool(name="w", bufs=nb))
        cpool = ctx.enter_context(tc.tile_pool(name="const", bufs=1))
        vpool = ctx.enter_context(tc.tile_pool(name="vec", bufs=2))
        epool = ctx.enter_context(tc.tile_pool(name="escr", bufs=2))
        pmain = ctx.enter_context(tc.tile_pool(name="pmain", bufs=3, space="PSUM"))
        pmisc = ctx.enter_context(tc.tile_pool(name="pmisc", bufs=1, space="PSUM"))
        ploss = ctx.enter_context(tc.tile_pool(name="ploss", bufs=1, space="PSUM"))

        # margin-chain inputs go FIRST on the scalar (ACT) HWDGE queue so
        # their tiny transfers beat the weight-stream flood (they starve
        # for ~7us if issued after it); the explicit ACT-set preload
        # follows (walrus may auto-insert one more load - harmless here)
        scal = cpool.tile([128, 3 * T], f32, tag="scal")
        nc.scalar.dma_start(out=scal[:], in_=scal_ext[:])
        idn = cpool.tile([128, 128], bf16, tag="idn")
        nc.scalar.dma_start(out=idn[:], in_=idn_ext[:])
        nc.scalar.add_instruction(mybir.InstLoadActFuncSet(
            name="preload-actset-6", act_func_set_id=6, ins=[], outs=[]))

        # sync (HWDGE) queue: xt/wtar first, then the weight stream.  The
        # SDMA engines round-robin between all queued transfers, so an
        # unthrottled stream makes band 0 finish nearly last.
        xtw = cpool.tile([128, 2, 256 * T], fp8, tag="xtw")
        nc.sync.dma_start(out=xtw[:], in_=xtw_ext[:])
        # chain the stream pairwise through 1-element gpsimd copies: band
        # b's DMA is forced to wait until band b-2 has fully landed, so at
        # most ~2 transfers share the channel and bands complete in
        # consumption order at full rate (an unthrottled stream makes all 8
        # crawl together; a pool-reuse throttle couples DMA to PE progress
        # and locksteps)
        w_tiles = [wpool.tile([128, 2, C], fp8, tag="wt", name=f"wt{b}")
                   for b in range(nb)]
        for b in range(nb):
            if b == 1:
                nc.gpsimd.tensor_copy(w_tiles[1][0:1, 0, 0:1],
                                      xtw[0:1, 0, 0:1])
            elif b >= 2:
                nc.gpsimd.tensor_copy(w_tiles[b][0:1, 0, 0:1],
                                      w_tiles[b - 2][0:1, 0, 0:1])
            nc.sync.dma_start(out=w_tiles[b][:], in_=wt_ext[b])

        loss_ps = ploss.tile([1, 1], f32, tag="loss")
        # one misc PSUM bank: cols 0..255 = per-tile target-logit matmuls,
        # cols 256..447 = warm-up dummy target
        dtar = pmisc.tile([128, 448], f32, tag="dtar")

        # PE warm-up: zero dummy matmuls keep the PE busy from the end of
        # the preamble so the HAM clock gate is already 8/8 (2.4 GHz) when
        # the first weight band lands
        jl = cpool.tile([128, NG], bf16, tag="jl")
        nc.vector.memset(jl[:], 0.0)
        jr = cpool.tile([128, 192], bf16, tag="jr")
        nc.vector.memset(jr[:], 0.0)
        for i in range(12):
            nc.tensor.matmul(dtar[0:NG, 256:448], jl[:], jr[:], start=True,
                             stop=True, tile_position=(0, 0))

        # target logits: ONE DoubleRow matmul per tile (stationary = all
        # 128 samples, moving = the 128 host-gathered target columns);
        # the diagonal holds <x_m, w_target_m>
        for t in range(T):
            dcol = slice(128 * (t % 2), 128 * (t % 2) + 128)
            nc.tensor.matmul(
                dtar[:, dcol],
                xtw[:, :, 128 * t:128 * (t + 1)],
                xtw[:, :, 128 * (T + t):128 * (T + t + 1)],
                start=True, stop=True, perf_mode=DR, tile_position=(0, 0),
            )

        # margin chain for ALL tiles batched on [128, T] tiles, entirely on
        # DVE (sqrt via Quake rsqrt) so nothing queues behind the big exps
        # on the scalar engine
        tcos = vpool.tile([128, T], f32, tag="tcos")
        for t in range(T):
            dcol = slice(128 * (t % 2), 128 * (t % 2) + 128)
            dmul = vpool.tile([128, 128], f32, tag="dmul", name=f"dmul{t}")
            nc.vector.tensor_tensor(dmul[:], dtar[:, dcol], idn[:], A.mult)
            nc.vector.reduce_sum(tcos[:, t:t + 1], dmul[:], axis=mybir.AxisListType.X)
        nc.vector.tensor_tensor(tcos[:], tcos[:], scal[:, T:2 * T], A.mult)
        om = vpool.tile([128, T], f32, tag="om")
        nc.vector.tensor_tensor(om[:], tcos[:], tcos[:], A.mult)
        nc.vector.tensor_scalar(om[:], om[:], -1.0, 1.0, op0=A.mult, op1=A.add)
        nc.vector.tensor_scalar_max(om[:], om[:], 1e-12)
        # sint = om * rsqrt(om): Quake seed + 2 Newton iterations
        yrs = vpool.tile([128, T], f32, tag="yrs")
        yi = yrs.bitcast(i32)
        nc.vector.tensor_scalar(yi[:], om.bitcast(i32)[:], 1, None,
                                op0=A.arith_shift_right)
        nc.vector.tensor_scalar(yi[:], yi[:], -1, 0x5F3759DF, op0=A.mult, op1=A.add)
        hz = vpool.tile([128, T], f32, tag="hz")
        nc.vector.tensor_scalar_mul(hz[:], om[:], 0.5)
        y2 = vpool.tile([128, T], f32, tag="y2")
        for _ in range(2):
            nc.vector.tensor_tensor(y2[:], yrs[:], yrs[:], A.mult)
            nc.vector.tensor_tensor(y2[:], y2[:], hz[:], A.mult)
            nc.vector.tensor_scalar(y2[:], y2[:], -1.0, 1.5, op0=A.mult, op1=A.add)
            nc.vector.tensor_tensor(yrs[:], yrs[:], y2[:], A.mult)
        sint = vpool.tile([128, T], f32, tag="sint")
        nc.vector.tensor_tensor(sint[:], om[:], yrs[:], A.mult)
        ctm = vpool.tile([128, T], f32, tag="ctm")
        nc.vector.tensor_scalar_mul(ctm[:], tcos[:], COS_M)
        sm = vpool.tile([128, T], f32, tag="sm")
        nc.vector.tensor_scalar_mul(sm[:], sint[:], SIN_M)
        nc.vector.tensor_tensor(ctm[:], ctm[:], sm[:], A.subtract)
        tms = vpool.tile([128, T], f32, tag="tms")
        nc.vector.tensor_scalar_add(tms[:], tcos[:], -SINMM)
        gt = vpool.tile([128, T], i32, tag="gt")
        nc.vector.tensor_scalar(gt[:], tcos[:], THETA, None, op0=A.is_gt)
        ft = vpool.tile([128, T], f32, tag="ft")
        nc.vector.select(ft[:], gt[:], ctm[:], tms[:])
        tf = vpool.tile([128, 2 * T], f32, tag="tf")
        nc.vector.tensor_scalar_mul(tf[:, 0:T], tcos[:], SCALE)
        nc.vector.tensor_scalar_mul(tf[:, T:2 * T], ft[:], SCALE)
        eb = vpool.tile([128, 2 * T], f32, tag="eb")
        nc.scalar.activation(eb[:], tf[:], AF.Exp)

        # per-tile softmax-sum collector: cols 0..1 = exp-chunk sums,
        # col 2 = -exp(s*t), col 3 = +exp(s*ft)
        S_t = [cpool.tile([128, 4], f32, tag=f"S{t}", name=f"S{t}") for t in range(T)]
        se2 = cpool.tile([128, T], f32, tag="se2")
        for t in range(T):
            nc.vector.tensor_scalar_mul(S_t[t][:, 2:3], eb[:, t:t + 1], -1.0)
            nc.vector.tensor_copy(S_t[t][:, 3:4], eb[:, T + t:T + t + 1])

        for t in range(T):
            cps = [pmain.tile([128, 1024], f32, tag="cos", name=f"cos{t}_{h}")
                   for h in range(2)]
            # k outer within a band: the 4 column-chunk matmuls share one
            # stationary load (k-inner reloads a conflicting strip per mm)
            for j in range(BPT):
                b = BPT * t + j
                o = 128 * t + NG * j
                if j == 0:
                    for cc in range(NCC):
                        nc.tensor.matmul(
                            cps[cc // 2][NG * j:NG * (j + 1),
                                         512 * (cc % 2):512 * (cc % 2) + 512],
                            xtw[:, :, o:o + NG],
                            w_tiles[b][:, :, 512 * cc:512 * (cc + 1)],
                            start=True, stop=True, perf_mode=DR,
                            tile_position=(0, NG * j),
                        )
                else:
                    for k in range(2):
                        for cc in range(NCC):
                            nc.tensor.matmul(
                                cps[cc // 2][NG * j:NG * (j + 1),
                                             512 * (cc % 2):512 * (cc % 2) + 512],
                                xtw[:, k, o:o + NG],
                                w_tiles[b][:, k, 512 * cc:512 * (cc + 1)],
                                start=(k == 0), stop=(k == 1),
                                tile_position=(0, NG * j),
                            )
                            if j == BPT - 1 and k == 1 and cc % 2 == 1:
                                h = cc // 2
                                escr = epool.tile(
                                    [128, 1024], bf16, tag=f"escr{h}",
                                    name=f"escr{t}_{h}")
                                nc.scalar.activation(
                                    escr[:], cps[h][:], AF.Exp,
                                    scale=scal[:, t:t + 1],
                                    accum_out=S_t[t][:, h:h + 1],
                                )
                if t == 0 and j < BPT - 1:
                    # gap fillers: the first tile is DMA-gated band to band;
                    # idle PE re-throttles the HAM clock, so keep it busy
                    for i in range(5):
                        nc.tensor.matmul(dtar[0:NG, 256:448], jl[:], jr[:],
                                         start=True, stop=True,
                                         tile_position=(0, 0))
            nc.vector.reduce_sum(se2[:, t:t + 1], S_t[t][:],
                                 axis=mybir.AxisListType.X)
        lse = cpool.tile([128, T], f32, tag="lse")
        nc.scalar.activation(lse[:], se2[:], AF.Ln)
        lb = cpool.tile([128, T], f32, tag="lb")
        nc.vector.tensor_tensor(lb[:], lse[:], tf[:, T:2 * T], A.subtract)
        for t in range(T):
            nc.tensor.matmul(
                loss_ps[:], scal[:, 2 * T + t:2 * T + t + 1], lb[:, t:t + 1],
                start=(t == 0), stop=(t == T - 1),
            )

        loss_sb = cpool.tile([1, 1], f32, tag="losssb")
        nc.vector.tensor_copy(loss_sb[:], loss_ps[:])
        nc.sync.dma_start(out=out_ext[:], in_=loss_sb[:])

    nc.compile()
    return nc


def _pack(logits, labels, weight):
    """Route samples to the core owning their group; build per-core inputs."""
    logits = np.asarray(logits, dtype=np.float32)
    labels = np.asarray(labels).astype(np.int64)
    weight = np.asarray(weight, dtype=np.float32)

    group = (labels // C).astype(np.int64)
    local = (labels % C).astype(np.int64)
    core = group // GPC
    gl = group % GPC

    # host prep: pre-normalized fp8 weights (x16 for fp8 normal range),
    # E-major DoubleRow layout; per-sample 1/||x|| scales
    wn16 = weight * (16.0 / np.maximum(
        np.sqrt(np.einsum('gce,gce->gc', weight, weight)), 1e-12))[:, :, None]
    wn16 = wn16.astype(FP8)
    wnt = np.ascontiguousarray(
        wn16.reshape(G, C, 2, 128).transpose(0, 3, 2, 1))   # (G, 128, 2, C)
    xq = logits.astype(FP8)
    rinv = (1.0 / np.maximum(np.sqrt((logits * logits).sum(-1)), 1e-12)
            ).astype(np.float32)

    idn = np.zeros((128, 128), dtype=BF16)
    idn[np.arange(128), np.arange(128)] = 1.0

    # band assignment: per (core, local-group), ceil(count/NG) bands
    percg = [[np.nonzero((core == c) & (gl == g))[0] for g in range(GPC)]
             for c in range(NCORES)]
    nbands = [sum(max(1, -(-len(idx) // NG)) for idx in percg[c])
              for c in range(NCORES)]
    nb = max(nbands)
    nb = -(-nb // BPT) * BPT  # round up to full sample tiles
    T = nb // BPT

    in_maps = []
    for c in range(NCORES):
        bands = []
        for g in range(GPC):
            idx = percg[c][g]
            nslice = max(1, -(-len(idx) // NG))
            for s in range(nslice):
                bands.append((g, idx[s * NG:(s + 1) * NG]))
        while len(bands) < nb:
            bands.append((0, np.empty(0, dtype=np.int64)))

        wt = np.empty((nb, 128, 2, C), dtype=FP8)
        xtw = np.empty((128, 2, 256 * T), dtype=FP8)
        xt = xtw[:, :, :128 * T]
        wtar = xtw[:, :, 128 * T:]
        scal = np.zeros((128, 3 * T), dtype=np.float32)
        xs = np.zeros((128, E), dtype=FP8)
        ws = np.zeros((128, E), dtype=FP8)
        for t in range(T):
            xs[:] = 0
            ws[:] = 0
            for j in range(BPT):
                g, idx = bands[BPT * t + j]
                wt[BPT * t + j] = wnt[c * GPC + g]
                sl = slice(NG * j, NG * j + len(idx))
                xs[sl] = xq[idx]
                ws[sl] = wn16[c * GPC + g, local[idx]]
                scal[sl, t] = 4.0 * rinv[idx]
                scal[sl, T + t] = rinv[idx] / 16.0
                scal[sl, 2 * T + t] = 1.0 / B
            xt[:, :, 128 * t:128 * (t + 1)] = xs.reshape(128, 2, 128).transpose(2, 1, 0)
            wtar[:, :, 128 * t:128 * (t + 1)] = ws.reshape(128, 2, 128).transpose(2, 1, 0)
        in_maps.append({
            "wt": wt, "xtw": xtw, "idn": idn, "scal": scal,
        })
    return in_maps, nb


def _run(logits, labels, weight, trace=False, **kw):
    from concourse.bass_utils import run_bass_kernel_spmd

    in_maps, nb = _pack(logits, labels, weight)
    nc = _graph_cache.get(nb)
    if nc is None:
        nc = _build(nb)
        _graph_cache[nb] = nc
    res = run_bass_kernel_spmd(nc, in_maps, core_ids=list(range(NCORES)),
                               trace=trace, **kw)
    total = sum(float(res.results[i]["out"][0, 0]) for i in range(NCORES))
    return np.asarray(total, dtype=np.float32), res


def kernel(logits, labels, weight):
    loss, _ = _run(logits, labels, weight)
    return loss


# revision 28
# speedup vs baseline: 1.2079x; 1.2079x over previous
"""ArcFace multi-head-sharded loss on 8 TRN2 NeuronCores.

Strategy: shard the (64, 2048, 256) weight table over the group axis —
each core owns 8 groups. Samples are routed host-side to the core owning
their group (the host routing replaces the all-to-all). The host also
pre-normalizes weight rows (cos is scale-invariant in w, so w/||w|| is a
pure re-layout), scales by 16 and quantizes to fp8e4 — this halves HBM
traffic vs bf16 and enables DoubleRow matmuls (contraction of 256 = E in
a single pass, 2 fp8 elements per PE beat).

Each core:
  - streams its 8 pre-normalized weight groups (fp8, 512KB/band),
  - computes cos(b, c) = <x_b, w_c> via DoubleRow matmuls into PSUM
    (samples on PSUM partitions, classes on free dim),
  - extracts the target logit with a tiny per-band matmul against
    host-gathered target columns + diagonal mask,
  - applies the ArcFace margin (sqrt via exp(0.5 ln)) and the CE loss
    per sample on-device: exp with fused per-sample scale (folding
    1/||x||) and accumulation over classes, LSE correction for the
    margin target, weighted reduce to a single scalar via matmul,
  - returns one partial-loss scalar.

Host: sums the 8 scalars. Samples are packed into bands of NG=32
partition rows, one weight group per band, BPT=4 bands per 128-row tile.
"""

import sys
import numpy as np
import ml_dtypes

FP8 = ml_dtypes.float8_e4m3
BF16 = ml_dtypes.bfloat16

_TRN_REPO = "/opt/trn_rl_repo"
if _TRN_REPO not in sys.path:
    sys.path.insert(0, _TRN_REPO)

# problem config (hardcoded per spec)
B, E, G, C = 512, 256, 64, 2048
NCORES = 8
GPC = G // NCORES        # weight groups per core
NG = 32                  # sample slots per band
BPT = 128 // NG          # bands per 128-partition sample tile
NCC = C // 512           # 512-col psum chunks per group
SCALE = 64.0
MARGIN = 0.5
COS_M = float(np.cos(MARGIN))
SIN_M = float(np.sin(MARGIN))
THETA = float(np.cos(np.pi - MARGIN))
SINMM = float(np.sin(np.pi - MARGIN) * MARGIN)

_graph_cache = {}


def _build(nb):
    """Build the per-core Bass graph for nb weight bands (nb % BPT == 0)."""
    from contextlib import ExitStack
    import concourse.bacc as bacc
    import concourse.tile as tile
    from concourse import mybir

    f32 = mybir.dt.float32
    bf16 = mybir.dt.bfloat16
    fp8 = mybir.dt.float8e4
    i32 = mybir.dt.int32
    A = mybir.AluOpType
    AF = mybir.ActivationFunctionType
    DR = mybir.MatmulPerfMode.DoubleRow

    T = nb // BPT
    nc = bacc.Bacc(None)

    W = 256 * T + 256
    wt_ext = nc.declare_dram_parameter("wt", [nb, 128, 2, C], fp8, isOutput=False)
    # ONE aux transfer carries everything small, so nothing races the
    # weight stream: per plane j, cols 0..128T = xt, 128T..256T = wtar;
    # plane 0 tail bytes = idn ([128,128] bf16), plane 1 tail = scal
    # ([128, 3T] f32: sc4 | rx16 | redw columns)
    xtw_ext = nc.declare_dram_parameter("xtw", [128, 2, W], fp8, isOutput=False)
    out_ext = nc.declare_dram_parameter("out", [1, 1], f32, isOutput=True)

    with tile.TileContext(nc) as tc, ExitStack() as ctx:
        wpool = ctx.enter_context(tc.tile_pool(name="w", bufs=nb))
        cpool = ctx.enter_context(tc.tile_pool(name="const", bufs=1))
        vpool = ctx.enter_context(tc.tile_pool(name="vec", bufs=2))
        epool = ctx.enter_context(tc.tile_pool(name="escr", bufs=2))
        pmain = ctx.enter_context(tc.tile_pool(name="pmain", bufs=3, space="PSUM"))
        pmisc = ctx.enter_context(tc.tile_pool(name="pmisc", bufs=1, space="PSUM"))
        ploss = ctx.enter_context(tc.tile_pool(name="ploss", bufs=1, space="PSUM"))

        # one resident ACT table set (exp + ln)
        nc.scalar.add_instruction(mybir.InstLoadActFuncSet(
            name="preload-actset-6", act_func_set_id=6, ins=[], outs=[]))

        # sync (HWDGE) queue: the aux transfer first, then the weight
        # stream band by band
        xtw = cpool.tile([128, 2, W], fp8, tag="xtw")
        nc.sync.dma_start(out=xtw[:], in_=xtw_ext[:])
        idn = xtw.bitcast(bf16)[:, 0, 128 * T:128 * T + 128]
        scalf = xtw.bitcast(f32)
        scal = scalf[:, 1, 64 * T:64 * T + 3 * T]
        w_tiles = [wpool.tile([128, 2, C], fp8, tag="wt", name=f"wt{b}")
                   for b in range(nb)]
        for b in range(nb):
            nc.sync.dma_start(out=w_tiles[b][:], in_=wt_ext[b])

        loss_ps = ploss.tile([1, 1], f32, tag="loss")
        # one misc PSUM bank: cols 0..255 = per-tile target-logit matmuls,
        # cols 256..447 = warm-up dummy target
        dtar = pmisc.tile([128, 448], f32, tag="dtar")

        # PE warm-up: zero dummy matmuls keep the PE busy from the end of
        # the preamble so the HAM clock gate is already 8/8 (2.4 GHz) when
        # the first weight band lands
        jl = cpool.tile([128, NG], bf16, tag="jl")
        nc.vector.memset(jl[:], 0.0)
        jr = cpool.tile([128, 192], bf16, tag="jr")
        nc.vector.memset(jr[:], 0.0)
        for i in range(12):
            nc.tensor.matmul(dtar[0:NG, 256:448], jl[:], jr[:], start=True,
                             stop=True, tile_position=(0, 0))

        # target logits: ONE DoubleRow matmul per tile (stationary = all
        # 128 samples, moving = the 128 host-gathered target columns);
        # the diagonal holds <x_m, w_target_m>
        for t in range(T):
            dcol = slice(128 * (t % 2), 128 * (t % 2) + 128)
            nc.tensor.matmul(
                dtar[:, dcol],
                xtw[:, :, 128 * t:128 * (t + 1)],
                xtw[:, :, 128 * (T + t):128 * (T + t + 1)],
                start=True, stop=True, perf_mode=DR, tile_position=(0, 0),
            )

        # margin chain for ALL tiles batched on [128, T] tiles, entirely on
        # DVE (sqrt via Quake rsqrt) so nothing queues behind the big exps
        # on the scalar engine
        tcos = vpool.tile([128, T], f32, tag="tcos")
        for t in range(T):
            dcol = slice(128 * (t % 2), 128 * (t % 2) + 128)
            dmul = vpool.tile([128, 128], f32, tag="dmul", name=f"dmul{t}")
            nc.vector.tensor_tensor(dmul[:], dtar[:, dcol], idn, A.mult)
            nc.vector.reduce_sum(tcos[:, t:t + 1], dmul[:], axis=mybir.AxisListType.X)
        nc.vector.tensor_tensor(tcos[:], tcos[:], scal[:, T:2 * T], A.mult)
        om = vpool.tile([128, T], f32, tag="om")
        nc.vector.tensor_tensor(om[:], tcos[:], tcos[:], A.mult)
        nc.vector.tensor_scalar(om[:], om[:], -1.0, 1.0, op0=A.mult, op1=A.add)
        nc.vector.tensor_scalar_max(om[:], om[:], 1e-12)
        # sint = om * rsqrt(om): Quake seed + 2 Newton iterations
        yrs = vpool.tile([128, T], f32, tag="yrs")
        yi = yrs.bitcast(i32)
        nc.vector.tensor_scalar(yi[:], om.bitcast(i32)[:], 1, None,
                                op0=A.arith_shift_right)
        nc.vector.tensor_scalar(yi[:], yi[:], -1, 0x5F3759DF, op0=A.mult, op1=A.add)
        hz = vpool.tile([128, T], f32, tag="hz")
        nc.vector.tensor_scalar_mul(hz[:], om[:], 0.5)
        y2 = vpool.tile([128, T], f32, tag="y2")
        for _ in range(2):
            nc.vector.tensor_tensor(y2[:], yrs[:], yrs[:], A.mult)
            nc.vector.tensor_tensor(y2[:], y2[:], hz[:], A.mult)
            nc.vector.tensor_scalar(y2[:], y2[:], -1.0, 1.5, op0=A.mult, op1=A.add)
            nc.vector.tensor_tensor(yrs[:], yrs[:], y2[:], A.mult)
        sint = vpool.tile([128, T], f32, tag="sint")
        nc.vector.tensor_tensor(sint[:], om[:], yrs[:], A.mult)
        ctm = vpool.tile([128, T], f32, tag="ctm")
        nc.vector.tensor_scalar_mul(ctm[:], tcos[:], COS_M)
        sm = vpool.tile([128, T], f32, tag="sm")
        nc.vector.tensor_scalar_mul(sm[:], sint[:], SIN_M)
        nc.vector.tensor_tensor(ctm[:], ctm[:], sm[:], A.subtract)
        tms = vpool.tile([128, T], f32, tag="tms")
        nc.vector.tensor_scalar_add(tms[:], tcos[:], -SINMM)
        gt = vpool.tile([128, T], i32, tag="gt")
        nc.vector.tensor_scalar(gt[:], tcos[:], THETA, None, op0=A.is_gt)
        ft = vpool.tile([128, T], f32, tag="ft")
        nc.vector.select(ft[:], gt[:], ctm[:], tms[:])
        tf = vpool.tile([128, 2 * T], f32, tag="tf")
        nc.vector.tensor_scalar_mul(tf[:, 0:T], tcos[:], SCALE)
        nc.vector.tensor_scalar_mul(tf[:, T:2 * T], ft[:], SCALE)
        eb = vpool.tile([128, 2 * T], f32, tag="eb")
        nc.scalar.activation(eb[:], tf[:], AF.Exp)

        # per-tile softmax-sum collector: cols 0..1 = exp-chunk sums,
        # col 2 = -exp(s*t), col 3 = +exp(s*ft)
        S_t = [cpool.tile([128, 4], f32, tag=f"S{t}", name=f"S{t}") for t in range(T)]
        se2 = cpool.tile([128, T], f32, tag="se2")
        for t in range(T):
            nc.vector.tensor_scalar_mul(S_t[t][:, 2:3], eb[:, t:t + 1], -1.0)
            nc.vector.tensor_copy(S_t[t][:, 3:4], eb[:, T + t:T + t + 1])

        for t in range(T):
            cps = [pmain.tile([128, 1024], f32, tag="cos", name=f"cos{t}_{h}")
                   for h in range(2)]
            # k outer within a band: the 4 column-chunk matmuls share one
            # stationary load (k-inner reloads a conflicting strip per mm)
            for j in range(BPT):
                b = BPT * t + j
                o = 128 * t + NG * j
                if j == 0:
                    for cc in range(NCC):
                        nc.tensor.matmul(
                            cps[cc // 2][NG * j:NG * (j + 1),
                                         512 * (cc % 2):512 * (cc % 2) + 512],
                            xtw[:, :, o:o + NG],
                            w_tiles[b][:, :, 512 * cc:512 * (cc + 1)],
                            start=True, stop=True, perf_mode=DR,
                            tile_position=(0, NG * j),
                        )
                else:
                    for k in range(2):
                        for cc in range(NCC):
                            nc.tensor.matmul(
                                cps[cc // 2][NG * j:NG * (j + 1),
                                             512 * (cc % 2):512 * (cc % 2) + 512],
                                xtw[:, k, o:o + NG],
                                w_tiles[b][:, k, 512 * cc:512 * (cc + 1)],
                                start=(k == 0), stop=(k == 1),
                                tile_position=(0, NG * j),
                            )
                            if j == BPT - 1 and k == 1 and cc % 2 == 1:
                                h = cc // 2
                                escr = epool.tile(
                                    [128, 1024], bf16, tag=f"escr{h}",
                                    name=f"escr{t}_{h}")
                                nc.scalar.activation(
                                    escr[:], cps[h][:], AF.Exp,
                                    scale=scal[:, t:t + 1],
                                    accum_out=S_t[t][:, h:h + 1],
                                )
                if t == 0 and j < BPT - 1:
                    # gap fillers: the first tile is DMA-gated band to band;
                    # idle PE re-throttles the HAM clock, so keep it busy
                    for i in range(5):
                        nc.tensor.matmul(dtar[0:NG, 256:448], jl[:], jr[:],
                                         start=True, stop=True,
                                         tile_position=(0, 0))
            nc.vector.reduce_sum(se2[:, t:t + 1], S_t[t][:],
                                 axis=mybir.AxisListType.X)
        lse = cpool.tile([128, T], f32, tag="lse")
        nc.scalar.activation(lse[:], se2[:], AF.Ln)
        lb = cpool.tile([128, T], f32, tag="lb")
        nc.vector.tensor_tensor(lb[:], lse[:], tf[:, T:2 * T], A.subtract)
        for t in range(T):
            nc.tensor.matmul(
                loss_ps[:], scal[:, 2 * T + t:2 * T + t + 1], lb[:, t:t + 1],
                start=(t == 0), stop=(t == T - 1),
            )

        loss_sb = cpool.tile([1, 1], f32, tag="losssb")
        nc.vector.tensor_copy(loss_sb[:], loss_ps[:])
        nc.sync.dma_start(out=out_ext[:], in_=loss_sb[:])

    nc.compile()
    return nc


def _pack(logits, labels, weight):
    """Route samples to the core owning their group; build per-core inputs."""
    logits = np.asarray(logits, dtype=np.float32)
    labels = np.asarray(labels).astype(np.int64)
    weight = np.asarray(weight, dtype=np.float32)

    group = (labels // C).astype(np.int64)
    local = (labels % C).astype(np.int64)
    core = group // GPC
    gl = group % GPC

    # host prep: pre-normalized fp8 weights (x16 for fp8 normal range),
    # E-major DoubleRow layout; per-sample 1/||x|| scales
    wn16 = weight * (16.0 / np.maximum(
        np.sqrt(np.einsum('gce,gce->gc', weight, weight)), 1e-12))[:, :, None]
    wn16 = wn16.astype(FP8)
    wnt = np.ascontiguousarray(
        wn16.reshape(G, C, 2, 128).transpose(0, 3, 2, 1))   # (G, 128, 2, C)
    xq = logits.astype(FP8)
    rinv = (1.0 / np.maximum(np.sqrt((logits * logits).sum(-1)), 1e-12)
            ).astype(np.float32)

    idn = np.zeros((128, 128), dtype=BF16)
    idn[np.arange(128), np.arange(128)] = 1.0

    # band assignment: per (core, local-group), ceil(count/NG) bands
    percg = [[np.nonzero((core == c) & (gl == g))[0] for g in range(GPC)]
             for c in range(NCORES)]
    nbands = [sum(max(1, -(-len(idx) // NG)) for idx in percg[c])
              for c in range(NCORES)]
    nb = max(nbands)
    nb = -(-nb // BPT) * BPT  # round up to full sample tiles
    T = nb // BPT

    in_maps = []
    for c in range(NCORES):
        bands = []
        for g in range(GPC):
            idx = percg[c][g]
            nslice = max(1, -(-len(idx) // NG))
            for s in range(nslice):
                bands.append((g, idx[s * NG:(s + 1) * NG]))
        while len(bands) < nb:
            bands.append((0, np.empty(0, dtype=np.int64)))

        W = 256 * T + 256
        wt = np.empty((nb, 128, 2, C), dtype=FP8)
        xtw = np.zeros((128, 2, W), dtype=FP8)
        xt = xtw[:, :, :128 * T]
        wtar = xtw[:, :, 128 * T:256 * T]
        xtw.view(np.uint8)[:, 0, 256 * T:256 * T + 256] = idn.view(np.uint8)
        scal = np.zeros((128, 3 * T), dtype=np.float32)
        xs = np.zeros((128, E), dtype=FP8)
        ws = np.zeros((128, E), dtype=FP8)
        for t in range(T):
            xs[:] = 0
            ws[:] = 0
            for j in range(BPT):
                g, idx = bands[BPT * t + j]
                wt[BPT * t + j] = wnt[c * GPC + g]
                sl = slice(NG * j, NG * j + len(idx))
                xs[sl] = xq[idx]
                ws[sl] = wn16[c * GPC + g, local[idx]]
                scal[sl, t] = 4.0 * rinv[idx]
                scal[sl, T + t] = rinv[idx] / 16.0
                scal[sl, 2 * T + t] = 1.0 / B
            xt[:, :, 128 * t:128 * (t + 1)] = xs.reshape(128, 2, 128).transpose(2, 1, 0)
            wtar[:, :, 128 * t:128 * (t + 1)] = ws.reshape(128, 2, 128).transpose(2, 1, 0)
        xtw.view(np.uint8)[:, 1, 256 * T:256 * T + 12 * T] = scal.view(np.uint8)
        in_maps.append({"wt": wt, "xtw": xtw})
    return in_maps, nb


def _run(logits, labels, weight, trace=False, **kw):
    from concourse.bass_utils import run_bass_kernel_spmd

    in_maps, nb = _pack(logits, labels, weight)
    nc = _graph_cache.get(nb)
    if nc is None:
        nc = _build(nb)
        _graph_cache[nb] = nc
    res = run_bass_kernel_spmd(nc, in_maps, core_ids=list(range(NCORES)),
                               trace=trace, **kw)
    total = sum(float(res.results[i]["out"][0, 0]) for i in range(NCORES))
    return np.asarray(total, dtype=np.float32), res


def kernel(logits, labels, weight):
    loss, _ = _run(logits, labels, weight)
    return loss
